# revision 1
# baseline (speedup 1.0000x reference)
"""Trainium2 Bass kernel for nn_CELoss_4896262717859.

Computes, for each query column c = idx_node[k] of a sparse adjacency matrix
(diagonal zeroed), a cross-entropy-style loss over the "lower" (r < c) and
"upper" (r > c) neighbor sets:

    contrib_side(c) = [cnt>0 and poscnt==1] * (log(sum_r m exp(out_r)) - poslogit) / cnt

All per-column quantities are sums of the form sum_r adj[r,c] * w[r] for
w in {1, pos, pos*out, exp(out)} -> computed as tensor-engine matvecs with a
triangular split, per-column for ALL N columns, then gathered at idx_node on
the host (O(N+K) combine).

Sharding: columns split into 8 slabs of 1024 (one per core). Each core reads
its [8192 x 1024] int32 slab contiguously (memory roofline), casts to bf16,
and accumulates psum[12, 1024] stats = {L,U} x {ones, pos, pl_hi, pl_lo,
e_hi, e_lo}. The core's row order is rotated by 1024*core so the diagonal
block always falls in local row-tiles 0..7 -> one NEFF serves all cores; the
L/U routing of full tiles is data-driven via zero-padded weight variants.
"""

import numpy as np
import ml_dtypes

N = 8192
K = 4096
NCORES = 8
SLAB = N // NCORES        # 1024 columns per core
P = 128                   # partition / tile edge
NT = N // P               # 64 row tiles
TPC = SLAB // P           # 8 diagonal tiles per core
NW = 6                    # weights per side
M = 2 * NW                # 12 psum partitions (L half = 0:6, U half = 6:12)
MMN = 512                 # max matmul free size

BF16 = ml_dtypes.bfloat16

_BASS_CACHE = {}


def _build_bass():
    import concourse.tile as tile
    import concourse.mybir as mybir
    from concourse import bacc

    # Bacc (not raw Bass): its compile() runs generate_event_semaphores,
    # which splits multi-sem waits — TRN2 instructions hold at most one.
    nc = bacc.Bacc("TRN2")
    adj = nc.dram_tensor("adj", [N, SLAB], mybir.dt.int32, kind="ExternalInput")
    wmat = nc.dram_tensor(
        "wmat", [P, (NT + TPC) * M], mybir.dt.bfloat16, kind="ExternalInput"
    )
    masks = nc.dram_tensor("masks", [P, 2 * P], mybir.dt.bfloat16, kind="ExternalInput")
    stats = nc.dram_tensor("stats", [M, SLAB], mybir.dt.float32, kind="ExternalOutput")

    with tile.TileContext(nc) as tc:
        with (
            tc.tile_pool(name="singles", bufs=1) as singles,
            # bufs multiple of 8 matches the 8-queue HWDGE round-robin: the
            # slot-reuse predecessor of each adj DMA lands on the SAME queue,
            # so its WAW ordering is implicit and the DMA carries a single
            # sync-wait (the DMA ISA struct has room for only one).
            tc.tile_pool(name="io", bufs=8) as io_pool,
            tc.tile_pool(name="bf", bufs=6) as bf_pool,
            tc.tile_pool(name="diag", bufs=TPC) as diag_pool,
            tc.tile_pool(name="psum", bufs=1, space="PSUM") as psum_pool,
        ):
            # issue the first two adjacency DMAs before anything else so the
            # HBM-saturated stream (the critical path) starts ~1.3us earlier;
            # the small wmat/masks loads slot in behind them.
            pre = {}
            for j in range(2):
                t = io_pool.tile([P, SLAB], mybir.dt.int32, tag="adj_i")
                nc.sync.dma_start(out=t, in_=adj[j * P : (j + 1) * P, :])
                pre[j] = t

            wsb = singles.tile([P, (NT + TPC) * M], mybir.dt.bfloat16)
            nc.sync.dma_start(out=wsb, in_=wmat[:, :])
            msb_raw = singles.tile([P, 2 * P], mybir.dt.bfloat16)
            nc.sync.dma_start(out=msb_raw, in_=masks[:, :])
            # Re-produce the masks on DVE: the DVE TensorTensor ISA struct has
            # room for a single sync-wait, so the diag-mask multiplies must
            # only ever depend on DVE-produced operands (one self-sem wait).
            msb = singles.tile([P, 2 * P], mybir.dt.bfloat16)
            nc.vector.tensor_copy(msb, msb_raw)

            # one psum tile per 512-col bank: Tile's RAW deps are whole-tile,
            # so separate tiles let bank A's copy-out overlap bank B's final
            # matmuls
            accs = [
                psum_pool.tile(
                    [M, MMN], mybir.dt.float32, tag=f"acc{b}", name=f"acc{b}"
                )
                for b in range(SLAB // MMN)
            ]

            def wv(v):
                return wsb[:, v * M : (v + 1) * M]

            # start=True zeroes the ENTIRE psum bank(s) a matmul touches, so
            # (a) every matmul stays inside one 512-col bank, (b) exactly the
            # first matmul touching each bank carries start=True.
            bank_started = [False] * (SLAB // MMN)

            def mm_seg(w, rhs_slice, a, b, stop=False):
                bank = a // MMN
                assert b <= (bank + 1) * MMN
                nc.tensor.matmul(
                    accs[bank][:, a - bank * MMN : b - bank * MMN], w, rhs_slice,
                    start=not bank_started[bank], stop=stop,
                    skip_group_check=True,
                )
                bank_started[bank] = True

            def mm(w, rhs_full, a, b, stop=False):
                while a < b:
                    e = min(b, (a // MMN + 1) * MMN)
                    mm_seg(w, rhs_full[:, a:e], a, e, stop=stop)
                    a = e

            for j in range(NT):
                last = j == NT - 1
                if j in pre:
                    adj_i = pre.pop(j)
                else:
                    adj_i = io_pool.tile([P, SLAB], mybir.dt.int32, tag="adj_i")
                    if last:
                        # split the final load so its first half (and the
                        # bank-A matmul) overlaps the second half's transfer
                        nc.sync.dma_start(
                            out=adj_i[:, 0:MMN], in_=adj[j * P :, 0:MMN]
                        )
                        nc.sync.dma_start(
                            out=adj_i[:, MMN:], in_=adj[j * P :, MMN:]
                        )
                    else:
                        nc.sync.dma_start(out=adj_i, in_=adj[j * P : (j + 1) * P, :])
                adj_b = bf_pool.tile([P, SLAB], mybir.dt.bfloat16)
                if last:
                    # fine-grained pipeline on the final tile: shortest
                    # latency from last-byte-arrival to last matmul, with
                    # the final chunk halved again to 128 cols
                    bounds = [0, 256, 512, 768, 896, SLAB]
                    for s, e in zip(bounds[:-1], bounds[1:]):
                        nc.vector.tensor_copy(adj_b[:, s:e], adj_i[:, s:e])
                        mm(wv(j), adj_b, s, e, stop=(e == SLAB))
                    continue
                nc.vector.tensor_copy(adj_b, adj_i)

                if j < TPC:
                    WL, WU = wv(j), wv(NT + j)
                    c0, c1 = j * P, (j + 1) * P
                    mlo = diag_pool.tile([P, P], mybir.dt.bfloat16)
                    nc.vector.tensor_mul(mlo, adj_b[:, c0:c1], msb[:, 0:P])
                    mup = diag_pool.tile([P, P], mybir.dt.bfloat16)
                    nc.vector.tensor_mul(mup, adj_b[:, c0:c1], msb[:, P : 2 * P])
                    # full columns left of the diag block: rows > cols -> U
                    mm(WU, adj_b, 0, c0)
                    mm_seg(WL, mlo, c0, c1)
                    mm_seg(WU, mup, c0, c1)
                    # full columns right of the diag block: rows < cols -> L
                    mm(WL, adj_b, c1, SLAB)
                else:
                    mm(wv(j), adj_b, 0, SLAB, stop=last)

            # per-bank copy-out: bank A's copy/DMA overlap the final bank-B
            # matmul (ACT reads psum bank A while PE writes bank B); bank B's
            # copy is split across ACT and DVE so the two halves run in
            # parallel on the critical tail
            out_sb = singles.tile([M, SLAB], mybir.dt.float32)
            nc.scalar.copy(out_sb[:, 0:MMN], accs[0])
            nc.sync.dma_start(out=stats[:, 0:MMN], in_=out_sb[:, 0:MMN])
            half = MMN // 2
            nc.scalar.copy(out_sb[:, MMN : MMN + half], accs[1][:, 0:half])
            nc.vector.tensor_copy(out_sb[:, MMN + half :], accs[1][:, half:])
            nc.sync.dma_start(out=stats[:, MMN:], in_=out_sb[:, MMN:])

    nc.compile()
    return nc


def _split_bf16(v):
    hi = v.astype(BF16)
    lo = (v - hi.astype(np.float64)).astype(BF16)
    return hi, lo


def _host_prep(outputs, targets):
    """Per-row weight table Wside [N, 6] bf16 and per-core inputs."""
    out = np.asarray(outputs, np.float64).reshape(-1)
    pos = (np.asarray(targets).reshape(-1) != 0).astype(np.float64)
    pl_hi, pl_lo = _split_bf16(pos * out)
    e_hi, e_lo = _split_bf16(np.exp(out))
    wside = np.stack(
        [
            np.ones(N, BF16),
            pos.astype(BF16),
            pl_hi,
            pl_lo,
            e_hi,
            e_lo,
        ],
        axis=1,
    ).astype(BF16)  # [N, 6]

    # triangular masks for the diagonal 128-block (strict)
    ri = np.arange(P)[:, None]
    ci = np.arange(P)[None, :]
    masks = np.concatenate(
        [(ri < ci).astype(BF16), (ri > ci).astype(BF16)], axis=1
    )  # [128, 256]
    return wside, np.ascontiguousarray(masks)


def _build_wmat(wside, core):
    """Per-core weight variants [128, (64+8)*12] bf16.

    Variant j (j<64): weights for local row tile j (absolute tile (8*core+j)%64).
      j < 8  -> L-only variant (diag tiles; U-only twin stored at 64+j)
      j >= 8 -> single variant, L or U half per the tile's position vs the slab
    """
    w = np.zeros((P, NT + TPC, M), dtype=BF16)
    for j in range(NT):
        t = (TPC * core + j) % NT
        rows = wside[t * P : (t + 1) * P, :]  # [128, 6]
        if j < TPC:
            w[:, j, 0:NW] = rows
            w[:, NT + j, NW:M] = rows
        elif j < NT - TPC * core:
            w[:, j, NW:M] = rows  # rows above slab columns -> U
        else:
            w[:, j, 0:NW] = rows  # wrapped rows below slab columns -> L
    return np.ascontiguousarray(w.reshape(P, (NT + TPC) * M))


def _build_shard(node_adj, core):
    """Rotated column slab [N, SLAB] int32: local row rho = (abs_row - SLAB*core) mod N."""
    c0 = SLAB * core
    cols = node_adj[:, c0 : c0 + SLAB]
    if core == 0:
        return np.ascontiguousarray(cols, dtype=np.int32)
    return np.ascontiguousarray(
        np.concatenate([cols[c0:], cols[:c0]], axis=0), dtype=np.int32
    )


def _combine(stats_list, idx_node):
    """stats_list: per-core [12, SLAB] f32 -> scalar loss (f64 math)."""
    full = np.concatenate([np.asarray(s, np.float64) for s in stats_list], axis=1)

    def side_contrib(x):
        cnt, poscnt = x[0], x[1]
        poslogit = x[2] + x[3]
        sumexp = x[4] + x[5]
        valid = (cnt > 0.5) & (np.abs(poscnt - 1.0) < 0.25)
        lse = np.log(np.where(valid, np.maximum(sumexp, 1e-300), 1.0))
        return np.where(valid, (lse - poslogit) / np.maximum(cnt, 1.0), 0.0)

    contrib = side_contrib(full[0:NW]) + side_contrib(full[NW:M])
    idx = np.asarray(idx_node).reshape(-1).astype(np.int64)
    return np.array(contrib[idx].sum(), dtype=np.float32)


def _ensure_axon_hooks_stub():
    """bass_utils imports antenv.axon_hooks when tracing is requested via
    env; the module is absent on some images. Provide a no-op stub so the
    import never crashes (hook=None -> bass_utils skips tracing)."""
    import sys
    import types

    try:
        import antenv.axon_hooks  # noqa: F401
    except ImportError:
        mod = types.ModuleType("antenv.axon_hooks")
        state = {"hook": None}
        mod.set_axon_ntff_profile_hook = lambda h: state.__setitem__("hook", h)
        mod.get_axon_ntff_profile_hook = lambda: state["hook"]
        sys.modules["antenv.axon_hooks"] = mod


def _device_stats(in_maps):
    _ensure_axon_hooks_stub()
    from concourse.bass_utils import run_bass_kernel_spmd

    if "nc" not in _BASS_CACHE:
        _BASS_CACHE["nc"] = _build_bass()
    last_exc = None
    for attempt in range(4):
        try:
            res = run_bass_kernel_spmd(
                _BASS_CACHE["nc"], in_maps, core_ids=list(range(NCORES))
            )
            return [r["stats"] for r in res.results]
        except Exception as e:  # transient NRT/accelerator hiccups
            last_exc = e
            try:
                # a fresh PJRT client usually recovers a transiently
                # "unrecoverable" accelerator; mirrors a process restart
                import jax
                import jax.extend.backend as _jeb

                jax.clear_caches()
                _jeb.clear_backends()
            except Exception:
                pass
            import time

            time.sleep(2.0 * (attempt + 1))
    raise last_exc


def _sim_stats(in_maps):
    """Numpy emulation of the device kernel (same inputs), for logic validation."""
    outs = []
    for m in in_maps:
        adj = m["adj"].astype(np.float32)
        w = m["wmat"].reshape(P, NT + TPC, M).astype(np.float32)
        msk = m["masks"].astype(np.float32)
        lowm, upm = msk[:, 0:P], msk[:, P:]
        acc = np.zeros((M, SLAB), np.float32)
        for j in range(NT):
            tile = adj[j * P : (j + 1) * P, :]
            if j < TPC:
                WL, WU = w[:, j, :], w[:, NT + j, :]
                c0, c1 = j * P, (j + 1) * P
                acc[:, :c0] += WU.T @ tile[:, :c0]
                acc[:, c0:c1] += WL.T @ (tile[:, c0:c1] * lowm)
                acc[:, c0:c1] += WU.T @ (tile[:, c0:c1] * upm)
                acc[:, c1:] += WL.T @ tile[:, c1:]
            else:
                acc += w[:, j, :].T @ tile
        outs.append(acc)
    return outs


def kernel(outputs, targets, node_adj, idx_node, _simulate=False):
    node_adj = np.asarray(node_adj)
    wside, masks = _host_prep(outputs, targets)
    in_maps = [
        {
            "adj": _build_shard(node_adj, d),
            "wmat": _build_wmat(wside, d),
            "masks": masks,
        }
        for d in range(NCORES)
    ]
    stats = _sim_stats(in_maps) if _simulate else _device_stats(in_maps)
    return _combine(stats, idx_node)



# revision 4
# speedup vs baseline: 3.6658x; 3.6658x over previous
"""Trainium2 Bass kernel for nn_CELoss_4896262717859.

For each query column c = idx_node[k] of a sparse adjacency matrix (diagonal
zeroed), computes a CE-style loss over the "lower" (r < c) and "upper" (r > c)
neighbor sets:

    contrib_side(c) = [cnt>0 and poscnt==1] * (log(sum_r m exp(out_r)) - poslogit) / cnt

Only the K=4096 gathered columns are ever read (host gathers them while
sharding, per the sharding hint), as fp8 0/1 bytes: 4 MB per core instead of
the 32 MB int32 full-matrix slab.

Device work per core (512 columns): for each of 32 row-blocks J (256 rows),
a Double-FP8 matmul (fp8 pairs, 2 rows/lane/cycle) producing the per-block
partial sums P[J, {cnt, e_hi, e_lo}, k] in PSUM. The lower/upper split is NOT
done on device: because idx_node is sorted, lower(k) = prefix of P over
J < idx[k]//256 plus an in-block partial that the host computes exactly from
the 256-row window around the diagonal. upper(k) = the suffix, likewise.
Positive-row stats (poscnt/poslogit) touch only ~2% of rows and are computed
exactly on host. All device sums are of nonneg terms -> no cancellation.

Weights: w0 = 1 (cnt; exact in fp8/f32 accum), w1 = fp8(exp(out)),
w2 = fp8(exp(out) - w1) (hi/lo split -> ~0.4% relative error on sumexp,
far inside the 2e-2 gate).
"""

import numpy as np
import ml_dtypes

N = 8192
K = 4096
NCORES = 8
SLAB = K // NCORES        # 512 query columns per core
P = 128                   # partitions
NCHUNK = 8                # DMA chunks per core (1024 rows each)
TPC = 8                   # 128-row subtiles per chunk
NJ = 32                   # 256-row double-tiles (2 subtiles each)
JPC = 4                   # double-tiles per chunk
S = 3                     # stats per column: cnt, e_hi, e_lo
SW = 4                    # stat slots in the weight table (padded)

FP8 = ml_dtypes.float8_e4m3   # TRN FP8_EXP4 (max 240, has inf) matches this

_BASS_CACHE = {}


def _build_bass():
    import concourse.tile as tile
    import concourse.mybir as mybir
    from concourse import bacc

    nc = bacc.Bacc("TRN2")
    # host pre-arranged layout: [chunk, partition, subtile, col] so each
    # chunk DMA reads 4 KB contiguous per partition
    adj = nc.dram_tensor(
        "adj", [NCHUNK, P, TPC, SLAB], mybir.dt.float8e4, kind="ExternalInput"
    )
    # weight table [p, pair, J, slot]: row 256J+128i+p, slots {1, e_hi, e_lo, 0}
    wts = nc.dram_tensor("wts", [P, 2, NJ, SW], mybir.dt.float8e4, kind="ExternalInput")
    # per-block partials, J-major so the two SBUF halves DMA out contiguously
    stats = nc.dram_tensor("stats", [NJ, S, SLAB], mybir.dt.float32, kind="ExternalOutput")

    DR = mybir.MatmulPerfMode.DoubleRow
    M = S * NJ  # 96 psum partitions: row 3J+s holds stat s of block J

    with tile.TileContext(nc) as tc:
        with (
            tc.tile_pool(name="singles", bufs=1) as singles,
            tc.tile_pool(name="io", bufs=NCHUNK - 1) as io_pool,
            tc.tile_pool(name="psum", bufs=1, space="PSUM") as psum_pool,
        ):
            # first adjacency chunk DMA goes first: it is the critical path
            pre = io_pool.tile([P, TPC, SLAB], mybir.dt.float8e4, tag="adj")
            nc.sync.dma_start(out=pre, in_=adj[0])

            # weight table rides the scalar (ACT) HWDGE ring so it does not
            # queue behind the adjacency stream on the sync ring
            wsb = singles.tile([P, 2, NJ, SW], mybir.dt.float8e4)
            nc.scalar.dma_start(out=wsb, in_=wts[:, :, :, :])

            # zero-padded stationary table: block J's weights sit at column
            # offset 3J of a 96-wide stationary, so every matmul accumulates
            # its 3 stat rows into psum partitions 3J..3J+3 of ONE bank and
            # the other 93 rows receive exact-zero products. Engine SBUF APs
            # must start at a 32-aligned partition, so this free-dim offset
            # construction is the only layout that both engines and a single
            # wide output DMA can address.
            wpad = singles.tile([P, 2, NJ, M], mybir.dt.float8e4)
            nc.vector.memset(wpad[:, :, 0 : NJ // 2, :], 0.0)
            nc.gpsimd.memset(wpad[:, :, NJ // 2 :, :], 0.0)
            for J in range(NJ):
                src = wsb[:, :, J : J + 1, 0:S]
                dst = wpad[:, :, J : J + 1, S * J : S * J + S]
                if J < NJ // 2:
                    nc.vector.tensor_copy(dst, src)
                else:
                    nc.scalar.copy(dst, src)

            pt = psum_pool.tile([M, SLAB], mybir.dt.float32)
            out_sb = singles.tile([M, SLAB], mybir.dt.float32)

            for c in range(NCHUNK):
                if c == 0:
                    parts = [(pre, 0, JPC)]
                elif c < NCHUNK - 1:
                    t = io_pool.tile([P, TPC, SLAB], mybir.dt.float8e4, tag="adj")
                    nc.sync.dma_start(out=t, in_=adj[c])
                    parts = [(t, 0, JPC)]
                else:
                    # split the final chunk so its first half computes while
                    # the second half is still in flight
                    t1 = io_pool.tile(
                        [P, TPC // 2, SLAB], mybir.dt.float8e4, tag="adjl", bufs=2
                    )
                    nc.sync.dma_start(out=t1, in_=adj[c, :, 0 : TPC // 2, :])
                    t2 = io_pool.tile(
                        [P, TPC // 2, SLAB], mybir.dt.float8e4, tag="adjl", bufs=2
                    )
                    nc.sync.dma_start(out=t2, in_=adj[c, :, TPC // 2 :, :])
                    parts = [(t1, 0, JPC // 2), (t2, JPC // 2, JPC // 2)]

                for tt, jbase, cnt in parts:
                    for jj in range(cnt):
                        J = JPC * c + jbase + jj
                        # P[3J+s, :] += sum_i wpad[:, i, J, :].T @ blk[:, i, :]
                        nc.tensor.matmul(
                            pt,
                            wpad[:, :, J : J + 1, :],
                            tt[:, 2 * jj : 2 * jj + 2, :],
                            start=(J == 0),
                            stop=(J == NJ - 1),
                            perf_mode=DR,
                        )

            nc.vector.tensor_copy(out_sb, pt)
            nc.scalar.dma_start(out=stats[:, :, :], in_=out_sb)

    nc.compile()
    return nc


def _host_prep(outputs):
    """Weight table [128, 2, 32, 4] fp8: row 256J + 128i + p."""
    out = np.asarray(outputs, np.float64).reshape(-1)
    e = np.exp(out)
    e_hi = e.astype(FP8)
    e_lo = (e - e_hi.astype(np.float64)).astype(FP8)

    def lay(v):  # [N] -> [P, 2, NJ]
        return v.reshape(NJ, 2, P).transpose(2, 1, 0)

    wts = np.zeros((P, 2, NJ, SW), FP8)
    wts[:, :, :, 0] = FP8(1.0)
    wts[:, :, :, 1] = lay(e_hi)
    wts[:, :, :, 2] = lay(e_lo)
    return np.ascontiguousarray(wts), e


def _build_shard(node_adj, idx, core):
    """fp8 gathered columns, [chunk, partition, subtile, col] layout."""
    cols = idx[core * SLAB : (core + 1) * SLAB]
    A8 = (node_adj[:, cols] != 0).astype(FP8)  # [N, SLAB], 0/1 exact
    return np.ascontiguousarray(
        A8.reshape(NCHUNK, TPC, P, SLAB).transpose(0, 2, 1, 3)
    )


def _build_in_maps(node_adj, idx, outputs):
    wts, e = _host_prep(outputs)
    in_maps = [
        {"adj": _build_shard(node_adj, idx, c), "wts": wts} for c in range(NCORES)
    ]
    return in_maps, e


def _sim_stats(in_maps):
    """Numpy emulation of the device kernel (same quantized inputs)."""
    outs = []
    for m in in_maps:
        adj = m["adj"].astype(np.float32)  # [chunk, p, t, k]
        w = m["wts"].astype(np.float32)    # [p, i, J, slot]
        st = np.zeros((NJ, S, SLAB), np.float32)
        for J in range(NJ):
            c, tbase = J // JPC, 2 * (J % JPC)
            for i in range(2):
                blk = adj[c, :, tbase + i, :]          # [p, k]
                st[J] += w[:, i, J, 0:S].T @ blk       # [S, k]
        outs.append(st)
    return outs


def _ensure_axon_hooks_stub():
    """bass_utils imports antenv.axon_hooks when tracing is requested via
    env; the module is absent on some images. Provide a no-op stub so the
    import never crashes (hook=None -> bass_utils skips tracing)."""
    import sys
    import types

    try:
        import antenv.axon_hooks  # noqa: F401
    except ImportError:
        mod = types.ModuleType("antenv.axon_hooks")
        state = {"hook": None}
        mod.set_axon_ntff_profile_hook = lambda h: state.__setitem__("hook", h)
        mod.get_axon_ntff_profile_hook = lambda: state["hook"]
        sys.modules["antenv.axon_hooks"] = mod


def _device_stats(in_maps):
    _ensure_axon_hooks_stub()
    from concourse.bass_utils import run_bass_kernel_spmd

    if "nc" not in _BASS_CACHE:
        _BASS_CACHE["nc"] = _build_bass()
    last_exc = None
    for attempt in range(4):
        try:
            res = run_bass_kernel_spmd(
                _BASS_CACHE["nc"], in_maps, core_ids=list(range(NCORES))
            )
            return [r["stats"] for r in res.results]
        except Exception as e:  # transient NRT/accelerator hiccups
            last_exc = e
            try:
                # a fresh PJRT client usually recovers a transiently
                # "unrecoverable" accelerator; mirrors a process restart
                import jax
                import jax.extend.backend as _jeb

                jax.clear_caches()
                _jeb.clear_backends()
            except Exception:
                pass
            import time

            time.sleep(2.0 * (attempt + 1))
    raise last_exc


def _combine(stats_list, node_adj, outputs, targets, idx, e):
    """Per-core [NJ, S, SLAB] f32 partials -> scalar loss (f64 math)."""
    out = np.asarray(outputs, np.float64).reshape(-1)
    Pf = np.concatenate(
        [np.asarray(s, np.float64) for s in stats_list], axis=2
    )  # [NJ, S, K]
    cnt_P = Pf[:, 0, :]
    se_P = Pf[:, 1, :] + Pf[:, 2, :]

    kk = np.arange(K)
    zero = np.zeros((1, K))
    cum_cnt = np.concatenate([zero, np.cumsum(cnt_P, axis=0)], axis=0)  # [NJ+1, K]
    cum_se = np.concatenate([zero, np.cumsum(se_P, axis=0)], axis=0)

    t2 = idx // 256
    pre_cnt = cum_cnt[t2, kk]
    pre_se = cum_se[t2, kk]
    suf_cnt = cum_cnt[NJ] - cum_cnt[t2 + 1, kk]
    suf_se = cum_se[NJ] - cum_se[t2 + 1, kk]

    # exact in-block window (256 rows around the diagonal crossover)
    d = (idx % 256).astype(np.int64)
    rows = (idx - d)[:, None] + np.arange(256)[None, :]      # [K, 256]
    W = node_adj[rows, idx[:, None]] != 0
    dr = np.arange(256)[None, :]
    W &= dr != d[:, None]                                    # drop diagonal
    e_win = e[rows]
    wlow = W & (dr < d[:, None])
    wup = W & (dr > d[:, None])
    lower_cnt = pre_cnt + wlow.sum(1)
    upper_cnt = suf_cnt + wup.sum(1)
    lower_se = pre_se + (e_win * wlow).sum(1)
    upper_se = suf_se + (e_win * wup).sum(1)

    # exact positive-row stats (~2% of rows)
    prows = np.flatnonzero(np.asarray(targets).reshape(-1) != 0)
    Ap = node_adj[np.ix_(prows, idx)] != 0                   # [npos, K]
    Ap &= prows[:, None] != idx[None, :]
    plow = prows[:, None] < idx[None, :]
    poscnt_low = (Ap & plow).sum(0)
    poscnt_up = (Ap & ~plow).sum(0)
    poslogit_low = (out[prows, None] * (Ap & plow)).sum(0)
    poslogit_up = (out[prows, None] * (Ap & ~plow)).sum(0)

    def side(cnt, se, poscnt, poslogit):
        valid = (poscnt == 1) & (cnt > 0.5)
        lse = np.log(np.where(valid, np.maximum(se, 1e-300), 1.0))
        return np.where(valid, (lse - poslogit) / np.maximum(cnt, 1.0), 0.0).sum()

    loss = side(lower_cnt, lower_se, poscnt_low, poslogit_low) + side(
        upper_cnt, upper_se, poscnt_up, poslogit_up
    )
    return np.float32(loss)


def kernel(outputs, targets, node_adj, idx_node, _simulate=False):
    node_adj = np.asarray(node_adj)
    idx = np.asarray(idx_node).reshape(-1).astype(np.int64)
    in_maps, e = _build_in_maps(node_adj, idx, outputs)
    stats = _sim_stats(in_maps) if _simulate else _device_stats(in_maps)
    return _combine(stats, node_adj, outputs, targets, idx, e)


# revision 8
# speedup vs baseline: 3.8458x; 1.0491x over previous
"""Trainium2 Bass kernel for nn_CELoss_4896262717859.

For each query column c = idx_node[k] of a sparse adjacency matrix (diagonal
zeroed), computes a CE-style loss over the "lower" (r < c) and "upper" (r > c)
neighbor sets:

    contrib_side(c) = [cnt>0 and poscnt==1] * (log(sum_r m exp(out_r)) - poslogit) / cnt

Only the gathered columns are ever read (host gathers them while sharding, per
the sharding hint) and duplicate idx_node entries are deduplicated, as fp8 0/1
bytes: ~3.4 MB per core instead of the 32 MB int32 full-matrix slab.

Device work per core (KC/8 columns): for each of 32 row-blocks J (256 rows),
a Double-FP8 matmul (fp8 pairs, 2 rows/lane/cycle) accumulating the per-block
partials P[J, {cnt, e_hi, e_lo}, k] into psum partitions 3J..3J+3 of a single
bank, using a zero-padded 96-wide stationary (block J's weights at column
offset 3J). The lower/upper split is NOT done on device: because columns are
sorted, lower(k) = prefix of P over J < c_k//256 plus an in-block partial the
host computes exactly from the 256-row window around the diagonal; upper(k) is
the suffix likewise. Positive-row stats (poscnt/poslogit) touch only ~2% of
rows and are host-exact. All device sums are of nonneg terms -> no
cancellation anywhere.

Weights: w0 = 1 (cnt; exact), w1 = fp8(exp(out)), w2 = fp8(exp(out) - w1)
(hi/lo split -> ~0.4% relative error on sumexp, far inside the 2e-2 gate).
"""

import numpy as np
import ml_dtypes

N = 8192
NCORES = 8
P = 128                   # partitions
NCHUNK = 8                # DMA chunks per core (1024 rows each)
TPC = 8                   # 128-row subtiles per chunk
NJ = 32                   # 256-row double-tiles (2 subtiles each)
JPC = 4                   # double-tiles per chunk
S = 3                     # stats per column: cnt, e_hi, e_lo
SW = 4                    # stat slots in the compact weight table
MH = S * NJ // 2          # 48 psum partitions per half: row 3(J%16)+s

SLAB_DEDUP = 416          # columns/core when unique(idx) fits (3328 total)
SLAB_FULL = 512           # fallback: all 4096 columns with duplicates

FP8 = ml_dtypes.float8_e4m3   # TRN FP8_EXP4 (max 240, has inf) matches this
BF16 = ml_dtypes.bfloat16

_BASS_CACHE = {}


def _build_bass(slab):
    import concourse.tile as tile
    import concourse.mybir as mybir
    from concourse import bacc

    nc = bacc.Bacc("TRN2")
    # host pre-arranged layout: [chunk, partition, subtile, col] so each
    # chunk DMA reads 8*slab contiguous bytes per partition
    adj = nc.dram_tensor(
        "adj", [NCHUNK, P, TPC, slab], mybir.dt.float8e4, kind="ExternalInput"
    )
    # compact weight table [p, pair, J, slot]: row 256J+128i+p
    wts = nc.dram_tensor("wts", [P, 2, NJ, SW], mybir.dt.float8e4, kind="ExternalInput")
    # per-block partials, J-major; bf16 (cnt <= 256 stays exact)
    stats = nc.dram_tensor(
        "stats", [NJ, S, slab], mybir.dt.bfloat16, kind="ExternalOutput"
    )

    DR = mybir.MatmulPerfMode.DoubleRow

    with tile.TileContext(nc) as tc:
        with (
            tc.tile_pool(name="singles", bufs=1) as singles,
            tc.tile_pool(name="io", bufs=NCHUNK - 1) as io_pool,
            tc.tile_pool(name="psum", bufs=1, space="PSUM") as psum_pool,
        ):
            # first adjacency chunk DMA goes first: it is the critical path
            pre = io_pool.tile([P, TPC, slab], mybir.dt.float8e4, tag="adj")
            nc.sync.dma_start(out=pre, in_=adj[0])

            # weight table rides the scalar (ACT) HWDGE ring so it does not
            # queue behind the adjacency stream on the sync ring
            wsb = singles.tile([P, 2, NJ, SW], mybir.dt.float8e4)
            nc.scalar.dma_start(out=wsb, in_=wts[:, :, :, :])

            # zero-padded stationary table: block J's weights sit at column
            # offset 3(J%16) of a 48-wide stationary, so each matmul
            # accumulates its 3 stat rows into psum partitions of one
            # half-bank (J 0..15 -> bank A, 16..31 -> bank B; bank A is
            # copied out + DMA'd while the PE still writes bank B). The
            # other 45 rows receive exact-zero products. Engine SBUF APs
            # must start at a 32-aligned partition, so this free-dim offset
            # construction is the only layout that both engines and a wide
            # output DMA can address.
            wpad = singles.tile([P, 2, NJ, MH], mybir.dt.float8e4)
            nc.vector.memset(wpad[:, :, 0 : NJ // 2, :], 0.0)
            nc.gpsimd.memset(wpad[:, :, NJ // 2 :, :], 0.0)
            for J in range(NJ):
                src = wsb[:, :, J : J + 1, 0:S]
                off = S * (J % (NJ // 2))
                dst = wpad[:, :, J : J + 1, off : off + S]
                if J < NJ // 2:
                    nc.vector.tensor_copy(dst, src)
                else:
                    nc.scalar.copy(dst, src)

            # pad psum tiles to full 2 KiB banks so each is bank-aligned
            pta = psum_pool.tile([MH, 512], mybir.dt.float32, padded_shape=None)
            ptb = psum_pool.tile([MH, 512], mybir.dt.float32, padded_shape=None)
            # half-split output so the first half leaves while matmuls run
            out_a = singles.tile([MH, slab], mybir.dt.bfloat16)
            out_b = singles.tile([MH, slab], mybir.dt.bfloat16)

            for c in range(NCHUNK):
                if c == 0:
                    parts = [(pre, 0, JPC)]
                elif c < NCHUNK - 1:
                    t = io_pool.tile([P, TPC, slab], mybir.dt.float8e4, tag="adj")
                    nc.sync.dma_start(out=t, in_=adj[c])
                    parts = [(t, 0, JPC)]
                else:
                    # split the final chunk 4-ways so each block's matmul
                    # runs while the next block is still in flight
                    parts = []
                    for q in range(JPC):
                        tq = io_pool.tile(
                            [P, 2, slab], mybir.dt.float8e4, tag="adjl", bufs=JPC
                        )
                        nc.sync.dma_start(
                            out=tq, in_=adj[c, :, 2 * q : 2 * q + 2, :]
                        )
                        parts.append((tq, q, 1))

                for tt, jbase, cnt in parts:
                    for jj in range(cnt):
                        J = JPC * c + jbase + jj
                        pt = pta if J < NJ // 2 else ptb
                        # P[3(J%16)+s] += sum_i wpad[:, i, J, :].T @ blk[:, i]
                        nc.tensor.matmul(
                            pt[:, 0:slab],
                            wpad[:, :, J : J + 1, :],
                            tt[:, 2 * jj : 2 * jj + 2, :],
                            start=(J % (NJ // 2) == 0),
                            stop=(J % (NJ // 2) == NJ // 2 - 1),
                            perf_mode=DR,
                        )
                        if J == NJ // 2 - 1:
                            # blocks 0..15 final: evacuate + DMA out early
                            # (bank A read runs beside bank B matmul writes)
                            nc.vector.tensor_copy(out_a, pta[:, 0:slab])
                            nc.scalar.dma_start(
                                out=stats[0 : NJ // 2], in_=out_a
                            )

            nc.vector.tensor_copy(out_b, ptb[:, 0:slab])
            nc.scalar.dma_start(out=stats[NJ // 2 :], in_=out_b)

    nc.compile()
    return nc


def _host_prep(outputs):
    """Compact weight table [128, 2, 32, 4] fp8: row 256J + 128i + p."""
    out = np.asarray(outputs, np.float64).reshape(-1)
    e = np.exp(out)
    e_hi = e.astype(FP8)
    e_lo = (e - e_hi.astype(np.float64)).astype(FP8)

    def lay(v):  # [N] -> [P, 2, NJ]
        return v.reshape(NJ, 2, P).transpose(2, 1, 0)

    wts = np.zeros((P, 2, NJ, SW), FP8)
    wts[:, :, :, 0] = FP8(1.0)
    wts[:, :, :, 1] = lay(e_hi)
    wts[:, :, :, 2] = lay(e_lo)
    return np.ascontiguousarray(wts), e


def _build_shard(node_adj, cols, core, slab):
    """fp8 gathered columns, [chunk, partition, subtile, col] layout."""
    cc = cols[core * slab : (core + 1) * slab]
    A8 = (node_adj[:, cc] != 0).astype(FP8)  # [N, slab], 0/1 exact
    return np.ascontiguousarray(
        A8.reshape(NCHUNK, TPC, P, slab).transpose(0, 2, 1, 3)
    )


def _prepare(node_adj, idx, outputs):
    """Choose dedup vs full columns; build per-core in_maps + combine ctx."""
    idxu = np.unique(idx)
    if idxu.size <= SLAB_DEDUP * NCORES:
        slab = SLAB_DEDUP
        cols = np.concatenate(
            [idxu, np.full(slab * NCORES - idxu.size, idxu[0], np.int64)]
        )
        mapk = np.searchsorted(idxu, idx)
    else:
        slab = SLAB_FULL
        cols = idx
        mapk = np.arange(idx.size)
    wts, e = _host_prep(outputs)
    in_maps = [
        {"adj": _build_shard(node_adj, cols, c, slab), "wts": wts}
        for c in range(NCORES)
    ]
    return in_maps, slab, cols, mapk, e


def _sim_stats(in_maps, slab):
    """Numpy emulation of the device kernel (same quantized inputs)."""
    outs = []
    for m in in_maps:
        adj = m["adj"].astype(np.float32)  # [chunk, p, t, k]
        w = m["wts"].astype(np.float32)    # [p, i, J, slot]
        st = np.zeros((NJ, S, slab), np.float32)
        for J in range(NJ):
            c, tbase = J // JPC, 2 * (J % JPC)
            for i in range(2):
                blk = adj[c, :, tbase + i, :]          # [p, k]
                st[J] += w[:, i, J, 0:S].T @ blk       # [S, k]
        outs.append(st.astype(BF16))
    return outs


def _ensure_axon_hooks_stub():
    """bass_utils imports antenv.axon_hooks when tracing is requested via
    env; the module is absent on some images. Provide a no-op stub so the
    import never crashes (hook=None -> bass_utils skips tracing)."""
    import sys
    import types

    try:
        import antenv.axon_hooks  # noqa: F401
    except ImportError:
        mod = types.ModuleType("antenv.axon_hooks")
        state = {"hook": None}
        mod.set_axon_ntff_profile_hook = lambda h: state.__setitem__("hook", h)
        mod.get_axon_ntff_profile_hook = lambda: state["hook"]
        sys.modules["antenv.axon_hooks"] = mod


def _device_stats(in_maps, slab):
    _ensure_axon_hooks_stub()
    from concourse.bass_utils import run_bass_kernel_spmd

    if slab not in _BASS_CACHE:
        _BASS_CACHE[slab] = _build_bass(slab)
    last_exc = None
    for attempt in range(4):
        try:
            res = run_bass_kernel_spmd(
                _BASS_CACHE[slab], in_maps, core_ids=list(range(NCORES))
            )
            return [r["stats"] for r in res.results]
        except Exception as e:  # transient NRT/accelerator hiccups
            last_exc = e
            try:
                # a fresh PJRT client usually recovers a transiently
                # "unrecoverable" accelerator; mirrors a process restart
                import jax
                import jax.extend.backend as _jeb

                jax.clear_caches()
                _jeb.clear_backends()
            except Exception:
                pass
            import time

            time.sleep(2.0 * (attempt + 1))
    raise last_exc


def _combine(stats_list, node_adj, outputs, targets, cols, mapk, e):
    """Per-core [NJ, S, slab] partials -> scalar loss (f64 math)."""
    out = np.asarray(outputs, np.float64).reshape(-1)
    Pf = np.concatenate(
        [np.asarray(s, np.float64) for s in stats_list], axis=2
    )  # [NJ, S, KC]
    KC = Pf.shape[2]
    cnt_P = Pf[:, 0, :]
    se_P = Pf[:, 1, :] + Pf[:, 2, :]

    kk = np.arange(KC)
    zero = np.zeros((1, KC))
    cum_cnt = np.concatenate([zero, np.cumsum(cnt_P, axis=0)], axis=0)  # [NJ+1, KC]
    cum_se = np.concatenate([zero, np.cumsum(se_P, axis=0)], axis=0)

    t2 = cols // 256
    pre_cnt = cum_cnt[t2, kk]
    pre_se = cum_se[t2, kk]
    suf_cnt = cum_cnt[NJ] - cum_cnt[t2 + 1, kk]
    suf_se = cum_se[NJ] - cum_se[t2 + 1, kk]

    # exact in-block window (256 rows around the diagonal crossover)
    d = (cols % 256).astype(np.int64)
    rows = (cols - d)[:, None] + np.arange(256)[None, :]     # [KC, 256]
    W = node_adj[rows, cols[:, None]] != 0
    dr = np.arange(256)[None, :]
    W &= dr != d[:, None]                                    # drop diagonal
    e_win = e[rows]
    wlow = W & (dr < d[:, None])
    wup = W & (dr > d[:, None])
    lower_cnt = pre_cnt + wlow.sum(1)
    upper_cnt = suf_cnt + wup.sum(1)
    lower_se = pre_se + (e_win * wlow).sum(1)
    upper_se = suf_se + (e_win * wup).sum(1)

    # exact positive-row stats (~2% of rows)
    prows = np.flatnonzero(np.asarray(targets).reshape(-1) != 0)
    Ap = node_adj[np.ix_(prows, cols)] != 0                  # [npos, KC]
    Ap &= prows[:, None] != cols[None, :]
    plow = prows[:, None] < cols[None, :]
    poscnt_low = (Ap & plow).sum(0)
    poscnt_up = (Ap & ~plow).sum(0)
    poslogit_low = (out[prows, None] * (Ap & plow)).sum(0)
    poslogit_up = (out[prows, None] * (Ap & ~plow)).sum(0)

    def side(cnt, se, poscnt, poslogit):
        valid = (poscnt == 1) & (cnt > 0.5)
        lse = np.log(np.where(valid, np.maximum(se, 1e-300), 1.0))
        return np.where(valid, (lse - poslogit) / np.maximum(cnt, 1.0), 0.0)

    contrib = side(lower_cnt, lower_se, poscnt_low, poslogit_low) + side(
        upper_cnt, upper_se, poscnt_up, poslogit_up
    )
    return np.float32(contrib[mapk].sum())


def kernel(outputs, targets, node_adj, idx_node, _simulate=False):
    node_adj = np.asarray(node_adj)
    idx = np.asarray(idx_node).reshape(-1).astype(np.int64)
    in_maps, slab, cols, mapk, e = _prepare(node_adj, idx, outputs)
    stats = _sim_stats(in_maps, slab) if _simulate else _device_stats(in_maps, slab)
    return _combine(stats, node_adj, outputs, targets, cols, mapk, e)


# revision 13
# speedup vs baseline: 4.2079x; 1.0942x over previous
"""Trainium2 Bass kernel for nn_CELoss_4896262717859.

For each query column c = idx_node[k] of a sparse adjacency matrix (diagonal
zeroed), computes a CE-style loss over the "lower" (r < c) and "upper" (r > c)
neighbor sets:

    contrib_side(c) = [cnt>0 and poscnt==1] * (log(sum_r m exp(out_r)) - poslogit) / cnt

Only the gathered columns are ever read (host gathers them while sharding, per
the sharding hint) and duplicate idx_node entries are deduplicated, as fp8 0/1
bytes: ~3.4 MB per core instead of the 32 MB int32 full-matrix slab.

Device work per core (KC/8 columns): for each of 32 row-blocks J (256 rows),
a Double-FP8 matmul (fp8 pairs, 2 rows/lane/cycle) accumulating the per-block
partials P[J, {cnt, e_hi, e_lo}, k] into psum partitions 3J..3J+3 of a single
bank, using a zero-padded 96-wide stationary (block J's weights at column
offset 3J). The lower/upper split is NOT done on device: because columns are
sorted, lower(k) = prefix of P over J < c_k//256 plus an in-block partial the
host computes exactly from the 256-row window around the diagonal; upper(k) is
the suffix likewise. Positive-row stats (poscnt/poslogit) touch only ~2% of
rows and are host-exact. All device sums are of nonneg terms -> no
cancellation anywhere.

Weights: w0 = 1 (cnt; exact), w1 = fp8(exp(out)), w2 = fp8(exp(out) - w1)
(hi/lo split -> ~0.4% relative error on sumexp, far inside the 2e-2 gate).
"""

import numpy as np
import ml_dtypes

N = 8192
NCORES = 8
P = 128                   # partitions
NCHUNK = 8                # DMA chunks per core (1024 rows each)
TPC = 8                   # 128-row subtiles per chunk
NJ = 32                   # 256-row double-tiles (2 subtiles each)
JPC = 4                   # double-tiles per chunk
S = 3                     # stats per column: cnt, e_hi, e_lo
SW = 4                    # stat slots in the compact weight table
MH = S * NJ // 2          # 48 psum partitions per half: row 3(J%16)+s

SLAB_DEDUP = 416          # columns/core when unique(idx) fits (3328 total)
SLAB_FULL = 512           # fallback: all 4096 columns with duplicates

FP8 = ml_dtypes.float8_e4m3   # TRN FP8_EXP4 (max 240, has inf) matches this
BF16 = ml_dtypes.bfloat16

_BASS_CACHE = {}


def _build_bass(slab):
    import concourse.tile as tile
    import concourse.mybir as mybir
    from concourse import bacc

    nc = bacc.Bacc("TRN2")
    # host pre-arranged layout: [chunk, partition, subtile*col] so each
    # chunk DMA moves TPC*slab contiguous bytes per partition in ONE
    # descriptor (3D [p, t, k] tiles emit per-(p,t) descriptors of `slab`
    # bytes, which sits below the 512 B DMA line-rate threshold)
    adj = nc.dram_tensor(
        "adj", [NCHUNK, P, TPC * slab], mybir.dt.float8e4, kind="ExternalInput"
    )
    # compact weight table [p, pair, J, slot]: row 256J+128i+p
    wts = nc.dram_tensor("wts", [P, 2, NJ, SW], mybir.dt.float8e4, kind="ExternalInput")
    # per-block partials, J-major; bf16 (cnt <= 256 stays exact)
    stats = nc.dram_tensor(
        "stats", [NJ, S, slab], mybir.dt.bfloat16, kind="ExternalOutput"
    )

    DR = mybir.MatmulPerfMode.DoubleRow

    with tile.TileContext(nc) as tc:
        with (
            tc.tile_pool(name="singles", bufs=1) as singles,
            tc.tile_pool(name="io", bufs=NCHUNK - 1) as io_pool,
            tc.tile_pool(name="psum", bufs=1, space="PSUM") as psum_pool,
        ):
            # first adjacency chunk DMA goes first: it is the critical path
            pre = io_pool.tile([P, TPC * slab], mybir.dt.float8e4, tag="adj")
            nc.sync.dma_start(out=pre, in_=adj[0])

            # weight table rides the scalar (ACT) HWDGE ring so it does not
            # queue behind the adjacency stream on the sync ring
            wsb = singles.tile([P, 2, NJ, SW], mybir.dt.float8e4)
            nc.scalar.dma_start(out=wsb, in_=wts[:, :, :, :])

            # zero-padded stationary table: block J's weights sit at column
            # offset 3(J%16) of a 48-wide stationary, so each matmul
            # accumulates its 3 stat rows into psum partitions of one
            # half-bank (J 0..15 -> bank A, 16..31 -> bank B; bank A is
            # copied out + DMA'd while the PE still writes bank B). The
            # other 45 rows receive exact-zero products. Engine SBUF APs
            # must start at a 32-aligned partition, so this free-dim offset
            # construction is the only layout that both engines and a wide
            # output DMA can address.
            wpad = singles.tile([P, 2, NJ, MH], mybir.dt.float8e4)
            nc.vector.memset(wpad[:, :, 0 : NJ // 2, :], 0.0)
            nc.gpsimd.memset(wpad[:, :, NJ // 2 :, :], 0.0)
            for J in range(NJ):
                src = wsb[:, :, J : J + 1, 0:S]
                off = S * (J % (NJ // 2))
                dst = wpad[:, :, J : J + 1, off : off + S]
                if J < NJ // 2:
                    nc.vector.tensor_copy(dst, src)
                else:
                    nc.scalar.copy(dst, src)

            # pad psum tiles to full 2 KiB banks so each is bank-aligned
            pta = psum_pool.tile([MH, 512], mybir.dt.float32, padded_shape=None)
            ptb = psum_pool.tile([MH, 512], mybir.dt.float32, padded_shape=None)
            # half-split output so the first half leaves while matmuls run
            out_a = singles.tile([MH, slab], mybir.dt.bfloat16)
            out_b = singles.tile([MH, slab], mybir.dt.bfloat16)

            for c in range(NCHUNK):
                if c == 0:
                    parts = [(pre, 0, JPC)]
                elif c < NCHUNK - 1:
                    t = io_pool.tile([P, TPC * slab], mybir.dt.float8e4, tag="adj")
                    nc.sync.dma_start(out=t, in_=adj[c])
                    parts = [(t, 0, JPC)]
                else:
                    # split the final chunk 4-ways so each block's matmul
                    # runs while the next block is still in flight
                    parts = []
                    for q in range(JPC):
                        tq = io_pool.tile(
                            [P, 2 * slab], mybir.dt.float8e4, tag="adjl", bufs=JPC
                        )
                        nc.sync.dma_start(
                            out=tq,
                            in_=adj[c, :, 2 * q * slab : (2 * q + 2) * slab],
                        )
                        parts.append((tq, q, 1))

                for tt, jbase, cnt in parts:
                    for jj in range(cnt):
                        J = JPC * c + jbase + jj
                        pt = pta if J < NJ // 2 else ptb
                        blk = tt[:, 2 * jj * slab : (2 * jj + 2) * slab].rearrange(
                            "p (i k) -> p i k", i=2
                        )
                        # P[3(J%16)+s] += sum_i wpad[:, i, J, :].T @ blk[:, i]
                        nc.tensor.matmul(
                            pt[:, 0:slab],
                            wpad[:, :, J : J + 1, :],
                            blk,
                            start=(J % (NJ // 2) == 0),
                            stop=(J % (NJ // 2) == NJ // 2 - 1),
                            perf_mode=DR,
                        )
                        if J == NJ // 2 - 1:
                            # blocks 0..15 final: evacuate + DMA out early
                            # (bank A read runs beside bank B matmul writes)
                            nc.vector.tensor_copy(out_a, pta[:, 0:slab])
                            nc.scalar.dma_start(
                                out=stats[0 : NJ // 2], in_=out_a
                            )

            nc.vector.tensor_copy(out_b, ptb[:, 0:slab])
            nc.scalar.dma_start(out=stats[NJ // 2 :], in_=out_b)

    nc.compile()
    return nc


def _host_prep(outputs):
    """Compact weight table [128, 2, 32, 4] fp8: row 256J + 128i + p."""
    out = np.asarray(outputs, np.float64).reshape(-1)
    e = np.exp(out)
    e_hi = e.astype(FP8)
    e_lo = (e - e_hi.astype(np.float64)).astype(FP8)

    def lay(v):  # [N] -> [P, 2, NJ]
        return v.reshape(NJ, 2, P).transpose(2, 1, 0)

    wts = np.zeros((P, 2, NJ, SW), FP8)
    wts[:, :, :, 0] = FP8(1.0)
    wts[:, :, :, 1] = lay(e_hi)
    wts[:, :, :, 2] = lay(e_lo)
    return np.ascontiguousarray(wts), e


def _build_shard(node_adj, cols, core, slab):
    """fp8 gathered columns, [chunk, partition, subtile*col] layout."""
    cc = cols[core * slab : (core + 1) * slab]
    A8 = (node_adj[:, cc] != 0).astype(FP8)  # [N, slab], 0/1 exact
    return np.ascontiguousarray(
        A8.reshape(NCHUNK, TPC, P, slab).transpose(0, 2, 1, 3).reshape(
            NCHUNK, P, TPC * slab
        )
    )


def _prepare(node_adj, idx, outputs):
    """Choose dedup vs full columns; build per-core in_maps + combine ctx."""
    idxu = np.unique(idx)
    if idxu.size <= SLAB_DEDUP * NCORES:
        slab = SLAB_DEDUP
        cols = np.concatenate(
            [idxu, np.full(slab * NCORES - idxu.size, idxu[0], np.int64)]
        )
        mapk = np.searchsorted(idxu, idx)
    else:
        slab = SLAB_FULL
        cols = idx
        mapk = np.arange(idx.size)
    wts, e = _host_prep(outputs)
    in_maps = [
        {"adj": _build_shard(node_adj, cols, c, slab), "wts": wts}
        for c in range(NCORES)
    ]
    return in_maps, slab, cols, mapk, e


def _sim_stats(in_maps, slab):
    """Numpy emulation of the device kernel (same quantized inputs)."""
    outs = []
    for m in in_maps:
        adj = m["adj"].astype(np.float32)  # [chunk, p, t*slab]
        w = m["wts"].astype(np.float32)    # [p, i, J, slot]
        st = np.zeros((NJ, S, slab), np.float32)
        for J in range(NJ):
            c, tbase = J // JPC, 2 * (J % JPC)
            for i in range(2):
                t = tbase + i
                blk = adj[c, :, t * slab : (t + 1) * slab]  # [p, k]
                st[J] += w[:, i, J, 0:S].T @ blk            # [S, k]
        outs.append(st.astype(BF16))
    return outs


def _ensure_axon_hooks_stub():
    """bass_utils imports antenv.axon_hooks when tracing is requested via
    env; the module is absent on some images. Provide a no-op stub so the
    import never crashes (hook=None -> bass_utils skips tracing)."""
    import sys
    import types

    try:
        import antenv.axon_hooks  # noqa: F401
    except ImportError:
        mod = types.ModuleType("antenv.axon_hooks")
        state = {"hook": None}
        mod.set_axon_ntff_profile_hook = lambda h: state.__setitem__("hook", h)
        mod.get_axon_ntff_profile_hook = lambda: state["hook"]
        sys.modules["antenv.axon_hooks"] = mod


def _device_stats(in_maps, slab):
    _ensure_axon_hooks_stub()
    from concourse.bass_utils import run_bass_kernel_spmd

    if slab not in _BASS_CACHE:
        _BASS_CACHE[slab] = _build_bass(slab)
    last_exc = None
    for attempt in range(4):
        try:
            res = run_bass_kernel_spmd(
                _BASS_CACHE[slab], in_maps, core_ids=list(range(NCORES))
            )
            return [r["stats"] for r in res.results]
        except Exception as e:  # transient NRT/accelerator hiccups
            last_exc = e
            try:
                # a fresh PJRT client usually recovers a transiently
                # "unrecoverable" accelerator; mirrors a process restart
                import jax
                import jax.extend.backend as _jeb

                jax.clear_caches()
                _jeb.clear_backends()
            except Exception:
                pass
            import time

            time.sleep(2.0 * (attempt + 1))
    raise last_exc


def _combine(stats_list, node_adj, outputs, targets, cols, mapk, e):
    """Per-core [NJ, S, slab] partials -> scalar loss (f64 math)."""
    out = np.asarray(outputs, np.float64).reshape(-1)
    Pf = np.concatenate(
        [np.asarray(s, np.float64) for s in stats_list], axis=2
    )  # [NJ, S, KC]
    KC = Pf.shape[2]
    cnt_P = Pf[:, 0, :]
    se_P = Pf[:, 1, :] + Pf[:, 2, :]

    kk = np.arange(KC)
    zero = np.zeros((1, KC))
    cum_cnt = np.concatenate([zero, np.cumsum(cnt_P, axis=0)], axis=0)  # [NJ+1, KC]
    cum_se = np.concatenate([zero, np.cumsum(se_P, axis=0)], axis=0)

    t2 = cols // 256
    pre_cnt = cum_cnt[t2, kk]
    pre_se = cum_se[t2, kk]
    suf_cnt = cum_cnt[NJ] - cum_cnt[t2 + 1, kk]
    suf_se = cum_se[NJ] - cum_se[t2 + 1, kk]

    # exact in-block window (256 rows around the diagonal crossover)
    d = (cols % 256).astype(np.int64)
    rows = (cols - d)[:, None] + np.arange(256)[None, :]     # [KC, 256]
    W = node_adj[rows, cols[:, None]] != 0
    dr = np.arange(256)[None, :]
    W &= dr != d[:, None]                                    # drop diagonal
    e_win = e[rows]
    wlow = W & (dr < d[:, None])
    wup = W & (dr > d[:, None])
    lower_cnt = pre_cnt + wlow.sum(1)
    upper_cnt = suf_cnt + wup.sum(1)
    lower_se = pre_se + (e_win * wlow).sum(1)
    upper_se = suf_se + (e_win * wup).sum(1)

    # exact positive-row stats (~2% of rows)
    prows = np.flatnonzero(np.asarray(targets).reshape(-1) != 0)
    Ap = node_adj[np.ix_(prows, cols)] != 0                  # [npos, KC]
    Ap &= prows[:, None] != cols[None, :]
    plow = prows[:, None] < cols[None, :]
    poscnt_low = (Ap & plow).sum(0)
    poscnt_up = (Ap & ~plow).sum(0)
    poslogit_low = (out[prows, None] * (Ap & plow)).sum(0)
    poslogit_up = (out[prows, None] * (Ap & ~plow)).sum(0)

    def side(cnt, se, poscnt, poslogit):
        valid = (poscnt == 1) & (cnt > 0.5)
        lse = np.log(np.where(valid, np.maximum(se, 1e-300), 1.0))
        return np.where(valid, (lse - poslogit) / np.maximum(cnt, 1.0), 0.0)

    contrib = side(lower_cnt, lower_se, poscnt_low, poslogit_low) + side(
        upper_cnt, upper_se, poscnt_up, poslogit_up
    )
    return np.float32(contrib[mapk].sum())


def kernel(outputs, targets, node_adj, idx_node, _simulate=False):
    node_adj = np.asarray(node_adj)
    idx = np.asarray(idx_node).reshape(-1).astype(np.int64)
    in_maps, slab, cols, mapk, e = _prepare(node_adj, idx, outputs)
    stats = _sim_stats(in_maps, slab) if _simulate else _device_stats(in_maps, slab)
    return _combine(stats, node_adj, outputs, targets, cols, mapk, e)
